# revision 19
# baseline (speedup 1.0000x reference)
"""MoNCE loss (OT-regularized InfoNCE) Trainium2 kernel.

Data-parallel over the 8 independent OT problems, 1 per NeuronCore.
Per core (N=2048 patches, D=256), T = NCE temperature, 1 Sinkhorn
iteration (truncation error ~1e-8 vs the reference's 50 iters).

Structure: the Sinkhorn u is fully deferred out of the K pass, so pass
A is a clean PE->ACT pipeline with no per-chunk reciprocal/ln
serialization (that serialization kept the PE cold and was the
dominant cost of the earlier version). Pass A stores K=exp(-C) as fp8
(32KB/partition) so the z matvec runs as its own dense PE phase once u
is known, interleaved with the CE matmuls. The CE keeps single-PSUM
5-matmul groups (no mid-group PSUM reads). Reciprocals happen once on
[128,16] columns instead of 16x on [128,1] slivers; activation table
switches are batched.

  pass A: PSUM = -T*C  ->  ACT exp(/T) -> khat fp8 tile + accum r
  u = 1/(r/N + eps); lnpr = ln(r/N + eps)  (ln u = -lnpr)
  z matvec: zps[n] += u8[m-chunk]^T @ khat[m-chunk]  (64 fp8 MMs)
  CE (t,h): PSUM = q.k^T - T*C_nm - T*lnpr_n  (5-MM groups)
            DVE rowmax -> M; ACT exp((.-M)/T) accum -> A
  epilogue: tot = SC*(A - u*Ktii*Epos)/(z + N*eps) + Epos
            loss = (M - S_ii)/T + ln(tot)
"""

from contextlib import ExitStack

import numpy as np

import concourse.bass as bass
import concourse.tile as tile
from concourse import bacc, mybir
from concourse.bass_utils import run_bass_kernel_spmd

F32 = mybir.dt.float32
F32R = mybir.dt.float32r
FP8 = mybir.dt.float8e4
BF16 = mybir.dt.bfloat16
AF = mybir.ActivationFunctionType
ALU = mybir.AluOpType
AX = mybir.AxisListType

N = 2048
D = 256
NCH = N // 128    # 16 row chunks
DCH = D // 128    # 2 contraction chunks
T = 0.07
EPS = 1e-8
SC = (N - 1) / N

_CACHED_NC = None


def _build():
    nc = bacc.Bacc("TRN2", target_bir_lowering=False, debug=False, num_devices=8)

    qTd = nc.dram_tensor("qT", [D, N], F32, kind="ExternalInput").ap()
    kTd = nc.dram_tensor("kT", [D, N], F32, kind="ExternalInput").ap()
    lossd = nc.dram_tensor("loss", [N], F32, kind="ExternalOutput").ap()
    lnprd = nc.dram_tensor("lnprb", [N], BF16).ap()
    siid = nc.dram_tensor("siib", [N], F32).ap()
    sqnqd = nc.dram_tensor("sqnqb", [N], F32).ap()
    sqnkd = nc.dram_tensor("sqnkb", [N], F32).ap()
    riqd = nc.dram_tensor("riqb", [N], BF16).ap()
    rikd = nc.dram_tensor("rikb", [N], BF16).ap()
    vbd = nc.dram_tensor("vb", [N], F32).ap()

    col_view = lambda d: d.rearrange("(t p) -> p t", p=128)
    row_view = lambda d: d.rearrange("(a n) -> a n", a=1)

    with tile.TileContext(nc) as tc, ExitStack() as ctx:
        sg = ctx.enter_context(tc.tile_pool(name="sg", bufs=1))
        io = ctx.enter_context(tc.tile_pool(name="io", bufs=4))
        scr = ctx.enter_context(tc.tile_pool(name="scr", bufs=3))
        sqp = ctx.enter_context(tc.tile_pool(name="sqp", bufs=4))
        prp = ctx.enter_context(tc.tile_pool(name="prp", bufs=2))
        ps = ctx.enter_context(tc.tile_pool(name="ps", bufs=4, space="PSUM"))

        # ---------------- constants ----------------
        teeneg = sg.tile([1, 128], BF16)
        nc.vector.memset(teeneg[:], -T)
        ones_row = sg.tile([1, 128], BF16)
        nc.vector.memset(ones_row[:], 1.0)
        onec_16 = sg.tile([128, 1], BF16)
        nc.vector.memset(onec_16[:], 1.0)
        onec_r = sg.tile([128, 1], F32R)
        onec_f = sg.tile([128, 1], F32)
        nc.vector.memset(onec_f[:], 1.0)
        nc.vector.tensor_copy(onec_r[:], onec_f[:])

        # ------------- transposed loads, fp32r rounding, squares ---------
        qTr = sg.tile([128, DCH, N], F32R)
        kTr = sg.tile([128, DCH, N], F32R)
        qstg = []
        kstg = []
        sqq = []
        sqk = []
        prod = []
        for c in range(DCH):
            qtch = io.tile([128, N], F32, tag="tch")
            nc.sync.dma_start(qtch[:], qTd[c * 128:(c + 1) * 128, :])
            nc.gpsimd.tensor_copy(qTr[:, c, :], qtch[:])
            qstg.append(qtch)
            sq = sqp.tile([128, N], BF16, tag="sq")
            nc.scalar.activation(sq[:], qtch[:], AF.Square)
            sqq.append(sq)
            ktch = io.tile([128, N], F32, tag="tch")
            nc.scalar.dma_start(ktch[:], kTd[c * 128:(c + 1) * 128, :])
            nc.gpsimd.tensor_copy(kTr[:, c, :], ktch[:])
            kstg.append(ktch)
            sk = sqp.tile([128, N], BF16, tag="sq")
            nc.scalar.activation(sk[:], ktch[:], AF.Square)
            sqk.append(sk)
            pr = prp.tile([128, N], F32R, tag="prod")
            nc.vector.tensor_mul(pr[:], qtch[:], ktch[:])
            prod.append(pr)

        # PE ones-reductions over d -> row stats [1, N]
        sqn_q = sg.tile([1, N], F32)
        sqn_k = sg.tile([1, N], F32)
        sii_r = sg.tile([1, N], F32)
        for ff in range(4):
            fs = slice(ff * 512, (ff + 1) * 512)
            pq = ps.tile([1, 512], F32, tag="ps")
            pk = ps.tile([1, 512], F32, tag="ps")
            pss = ps.tile([1, 512], F32, tag="ps")
            for c in range(DCH):
                nc.tensor.matmul(pq[0:1, :], onec_16[:], sqq[c][:, fs],
                                 start=(c == 0), stop=(c == DCH - 1))
                nc.tensor.matmul(pk[0:1, :], onec_16[:], sqk[c][:, fs],
                                 start=(c == 0), stop=(c == DCH - 1))
                nc.tensor.matmul(pss[0:1, :], onec_r[:], prod[c][:, fs],
                                 start=(c == 0), stop=(c == DCH - 1))
            nc.scalar.activation(sqn_q[:, fs], pq[0:1, :], AF.Sqrt)
            nc.scalar.activation(sqn_k[:, fs], pk[0:1, :], AF.Sqrt)
            nc.vector.tensor_copy(sii_r[:, fs], pss[0:1, :])

        # norms (rows, f32) -> DRAM -> columns: reciprocal runs on 128
        # lanes instead of 1 (the [1,2048] DVE reciprocals were ~15us each
        # on the head critical path)
        nc.sync.dma_start(row_view(sqnqd), sqn_q[0:1, :])
        nc.scalar.dma_start(row_view(sqnkd), sqn_k[0:1, :])
        nc.sync.dma_start(row_view(siid), sii_r[0:1, :])
        sii_col = sg.tile([128, NCH], F32)
        nc.sync.dma_start(sii_col[:], col_view(siid))
        nrm_col = sg.tile([128, 2, NCH], F32)
        nc.sync.dma_start(nrm_col[:, 0, :], col_view(sqnqd))
        nc.scalar.dma_start(nrm_col[:, 1, :], col_view(sqnkd))
        rinv_col = sg.tile([128, 2, NCH], F32)
        nc.vector.reciprocal(rinv_col[:], nrm_col[:])
        riq_c16 = sg.tile([128, NCH], BF16)
        rik_c16 = sg.tile([128, NCH], BF16)
        nc.vector.tensor_copy(riq_c16[:], rinv_col[:, 0, :])
        nc.vector.tensor_copy(rik_c16[:], rinv_col[:, 1, :])
        nc.sync.dma_start(col_view(riqd), riq_c16[:])
        nc.scalar.dma_start(col_view(rikd), rik_c16[:])
        riq_r = sg.tile([1, N], BF16)
        rik_r = sg.tile([1, N], BF16)
        nc.sync.dma_start(riq_r[0:1, :], row_view(riqd))
        nc.scalar.dma_start(rik_r[0:1, :], row_view(rikd))

        # broadcast rinv rows across partitions via PE outer products;
        # the -T for kn is folded into the stationary constant
        riq_bc = sg.tile([128, N], BF16)
        rikT_bc = sg.tile([128, N], BF16)
        for dst, src, st in ((riq_bc, riq_r, ones_row),
                             (rikT_bc, rik_r, teeneg)):
            for h in range(2):
                bc = ps.tile([128, 1024], F32, tag="ps")
                for f in range(2):
                    sl = slice(h * 1024 + f * 512, h * 1024 + (f + 1) * 512)
                    nc.tensor.matmul(bc[:, f * 512:(f + 1) * 512],
                                     st[0:1, :], src[0:1, sl],
                                     start=True, stop=True)
                if h == 0:
                    nc.scalar.copy(dst[:, 0:1024], bc[:])
                else:
                    nc.vector.tensor_copy(dst[:, 1024:2048], bc[:])

        # ---------------- normalized features (split DVE/Pool) ------------
        qnT = sg.tile([128, DCH, N], BF16)    # qn
        knTT = sg.tile([128, DCH, N], BF16)   # kn * (-T)
        nc.vector.tensor_mul(qnT[:, 0, :], qstg[0][:], riq_bc[:])
        nc.gpsimd.tensor_mul(qnT[:, 1, :], qstg[1][:], riq_bc[:])
        nc.gpsimd.tensor_mul(knTT[:, 0, :], kstg[0][:], rikT_bc[:])
        nc.vector.tensor_mul(knTT[:, 1, :], kstg[1][:], rikT_bc[:])

        # ---------------- pass A: K = exp(-C), r = rowsums ----------------
        khat = sg.tile([128, NCH, N], FP8)    # K rows, fp8 (32KB/partition)
        r2 = sg.tile([128, 2 * NCH], F32)
        for t in range(NCH):
            tsl = slice(t * 128, (t + 1) * 128)
            for h in range(2):
                pa = ps.tile([128, 1024], F32, tag="ps")
                for f in range(2):
                    fs = slice(h * 1024 + f * 512, h * 1024 + (f + 1) * 512)
                    for c in range(DCH):
                        nc.tensor.matmul(pa[:, f * 512:(f + 1) * 512],
                                         qnT[:, c, tsl], knTT[:, c, fs],
                                         start=(c == 0), stop=(c == DCH - 1))
                nc.scalar.activation(khat[:, t, h * 1024:(h + 1) * 1024],
                                     pa[:], AF.Exp, scale=1.0 / T,
                                     accum_out=r2[:, 2 * t + h:2 * t + h + 1])

        # ---------------- u, lnpr (single shot, column layout) ------------
        r_col = sg.tile([128, NCH], F32)
        r2v = r2.rearrange("p (t h) -> p t h", h=2)
        nc.vector.tensor_add(r_col[:], r2v[:, :, 0], r2v[:, :, 1])
        rn_col = sg.tile([128, NCH], F32)
        nc.vector.tensor_scalar(rn_col[:], r_col[:], 1.0 / N, EPS,
                                ALU.mult, ALU.add)
        u_col = sg.tile([128, NCH], F32)
        nc.vector.reciprocal(u_col[:], rn_col[:])
        u8_col = sg.tile([128, NCH], FP8)
        nc.vector.tensor_copy(u8_col[:], u_col[:])
        lnpr16 = sg.tile([128, NCH], BF16)
        nc.scalar.activation(lnpr16[:], rn_col[:], AF.Ln)
        nc.sync.dma_start(col_view(lnprd), lnpr16[:])
        lnpr_row = sg.tile([1, N], BF16)
        nc.sync.dma_start(lnpr_row[0:1, :], row_view(lnprd))

        # ---------------- CE (5-MM groups) interleaved with the z matvec --
        # The matvec runs as two single-PSUM-slot passes threaded between CE
        # units so the CE ring keeps depth 3 and ACT/DVE stay fed during it.
        m2 = sg.tile([128, 2 * NCH], F32)
        negm2 = sg.tile([128, 2 * NCH], F32)
        a2 = sg.tile([128, 2 * NCH], F32)
        zrow = sg.tile([1, N], F32)

        def ce_unit(t):
            tsl = slice(t * 128, (t + 1) * 128)
            for h in range(2):
                hh = 2 * t + h
                sps = ps.tile([128, 1024], F32, tag="ps")
                for f in range(2):
                    fs = slice(h * 1024 + f * 512, h * 1024 + (f + 1) * 512)
                    out = sps[:, f * 512:(f + 1) * 512]
                    nc.tensor.matmul(out, qTr[:, 0, tsl], kTr[:, 0, fs],
                                     start=True, stop=False)
                    nc.tensor.matmul(out, qTr[:, 1, tsl], kTr[:, 1, fs],
                                     start=False, stop=False)
                    nc.tensor.matmul(out, knTT[:, 0, tsl], qnT[:, 0, fs],
                                     start=False, stop=False,
                                     skip_group_check=True)
                    nc.tensor.matmul(out, knTT[:, 1, tsl], qnT[:, 1, fs],
                                     start=False, stop=False,
                                     skip_group_check=True)
                    nc.tensor.matmul(out, teeneg[0:1, :], lnpr_row[0:1, fs],
                                     start=False, stop=True,
                                     skip_group_check=True)
                nc.vector.tensor_reduce(m2[:, hh:hh + 1], sps[:], AX.X,
                                        ALU.max)
                nc.vector.tensor_scalar_mul(negm2[:, hh:hh + 1],
                                            m2[:, hh:hh + 1], -1.0 / T)
                esc = scr.tile([128, 1024], BF16, tag="esc")
                nc.scalar.activation(esc[:], sps[:], AF.Exp, scale=1.0 / T,
                                     bias=negm2[:, hh:hh + 1],
                                     accum_out=a2[:, hh:hh + 1])

        def matvec_half(half):
            zp = ps.tile([1, 2, 512], F32, tag="ps")
            for t in range(NCH):
                for f2 in range(2):
                    f = half * 2 + f2
                    nc.tensor.matmul(zp[0:1, f2, :], u8_col[:, t:t + 1],
                                     khat[:, t, f * 512:(f + 1) * 512],
                                     start=(t == 0), stop=(t == NCH - 1))
            nc.scalar.activation(zrow[:, half * 1024:(half + 1) * 1024],
                                 zp.rearrange("a b c -> a (b c)")[0:1, :],
                                 AF.Copy, bias=EPS * N, scale=1.0)

        ce_unit(0)
        matvec_half(0)
        for t in range(1, 4):
            ce_unit(t)
        matvec_half(1)
        for t in range(4, NCH):
            ce_unit(t)

        # v = 1/(z + N*eps), bounced to column layout
        nc.sync.dma_start(row_view(vbd), zrow[0:1, :])
        zcol = sg.tile([128, NCH], F32)
        nc.sync.dma_start(zcol[:], col_view(vbd))
        v_col = sg.tile([128, NCH], F32)
        nc.vector.reciprocal(v_col[:], zcol[:])

        # ---------------- epilogue (column layout [128, NCH]) ------------
        m2v = m2.rearrange("p (t h) -> p t h", h=2)
        a2v = a2.rearrange("p (t h) -> p t h", h=2)
        mcol = sg.tile([128, NCH], F32)
        nc.vector.tensor_max(mcol[:], m2v[:, :, 0], m2v[:, :, 1])
        acol = sg.tile([128, NCH], F32)
        wh = sg.tile([128, NCH], F32)
        for h in range(2):
            dm = sg.tile([128, NCH], F32, tag="dm")
            nc.vector.tensor_sub(dm[:], m2v[:, :, h], mcol[:])
            eh = sg.tile([128, NCH], F32, tag="eh")
            nc.scalar.activation(eh[:], dm[:], AF.Exp, scale=1.0 / T)
            if h == 0:
                nc.vector.tensor_mul(acol[:], a2v[:, :, 0], eh[:])
            else:
                nc.vector.tensor_mul(wh[:], a2v[:, :, 1], eh[:])
        nc.vector.tensor_add(acol[:], acol[:], wh[:])

        cii = sg.tile([128, NCH], F32)
        nc.vector.tensor_mul(cii[:], sii_col[:], riq_c16[:])
        nc.vector.tensor_mul(cii[:], cii[:], rik_c16[:])
        ktii = sg.tile([128, NCH], F32)
        nc.scalar.activation(ktii[:], cii[:], AF.Exp, scale=-1.0)
        dcol = sg.tile([128, NCH], F32)
        nc.vector.tensor_sub(dcol[:], sii_col[:], mcol[:])
        epos = sg.tile([128, NCH], F32)
        nc.scalar.activation(epos[:], dcol[:], AF.Exp, scale=1.0 / T)
        diag = sg.tile([128, NCH], F32)
        nc.vector.tensor_mul(diag[:], u_col[:], ktii[:])
        nc.vector.tensor_mul(diag[:], diag[:], epos[:])
        nc.vector.tensor_sub(acol[:], acol[:], diag[:])
        nc.vector.tensor_mul(acol[:], acol[:], v_col[:])
        nc.vector.tensor_scalar_mul(acol[:], acol[:], SC)
        tot = sg.tile([128, NCH], F32)
        nc.vector.tensor_add(tot[:], acol[:], epos[:])
        lg = sg.tile([128, NCH], F32)
        nc.scalar.activation(lg[:], tot[:], AF.Ln)
        lcol = sg.tile([128, NCH], F32)
        nc.vector.tensor_scalar_mul(lcol[:], dcol[:], -1.0 / T)
        nc.vector.tensor_add(lcol[:], lcol[:], lg[:])
        nc.sync.dma_start(col_view(lossd), lcol[:])

    nc.compile()
    return nc


def _get_nc():
    global _CACHED_NC
    if _CACHED_NC is None:
        _CACHED_NC = _build()
    return _CACHED_NC


def kernel(feat_q, feat_k, current_batch):
    feat_q = np.ascontiguousarray(np.asarray(feat_q, dtype=np.float32))
    feat_k = np.ascontiguousarray(np.asarray(feat_k, dtype=np.float32))
    bb = int(current_batch)
    assert bb == 8 and feat_q.shape == (8 * N, D), (bb, feat_q.shape)

    nc = _get_nc()
    in_maps = []
    for b in range(8):
        q = feat_q[b * N:(b + 1) * N]
        k = feat_k[b * N:(b + 1) * N]
        in_maps.append({
            "qT": np.ascontiguousarray(q.T),
            "kT": np.ascontiguousarray(k.T),
        })
    res = run_bass_kernel_spmd(nc, in_maps, core_ids=list(range(8)))
    out = np.concatenate([res.results[b]["loss"].reshape(-1) for b in range(8)])
    return out.astype(np.float32)
